# revision 19
# baseline (speedup 1.0000x reference)
"""TalkingHeadAttn Trainium2 kernel (8-core data parallel over batch).

Reference computation per batch element (B=8, N=1024, C=768, H=16, HD=48):
  qkv = x @ w_qkv.T + b_qkv ; split q,k,v per head
  S   = (q*scale) @ k.T                       [H,N,N]
  T   = einsum(S, w_l) (+ b_l, dropped: softmax-invariant)
  A   = softmax(T, axis=-1)
  P   = einsum(A, w_w) + b_w
  out = (P @ v) reshaped  @ w_proj.T + b_proj

Layout strategy (per core, one batch element):
  - qT/kT/vT computed "d-major" ([head-dim rows, tokens]) in head-pair padded
    tiles: tile t holds heads (2t, 2t+1) at partition ranges [0:48) and [64:112).
  - Scores computed transposed per (head, k-chunk): S^T [k=128, q=128] and
    stacked into ST_kc [k, (h,q)] bf16 tiles.
  - A PE gather-transpose converts ST into the "interleaved" layout
    S_hat[(q8,h), k] (8 queries x 16 heads on partitions), where both
    talking-heads mixes become single 128x128-contraction matmuls with
    host-built block-diagonal weight matrices, and softmax rows live on
    partitions with k in the free dim.
  - mix2 is fused with the P-transpose: P^T chunk = A_chunk.T @ BDw_scaled
    (per-row 1/sum softmax normalization folded into BDw as a partition scale).
  - attn@v per head via gathered P^T columns (M=128 queries), then PE-transpose
    of the output back to d-major for the projection, whose output is computed
    transposed so b_proj is a per-partition bias; host un-transposes.
  - b_l dropped (softmax shift invariance); b_w folded into a per-head
    constant correction b_w[g]*colsum(V) added as an OAT bias; b_v stays in V.
"""

import os
import sys
import functools

import numpy as np

for _p in ("/opt/trn_rl_repo", os.path.expanduser("~/.axon_site/_ro/trn_rl_repo")):
    if os.path.isdir(_p) and _p not in sys.path:
        sys.path.insert(0, _p)

import ml_dtypes

B, N, C = 8, 1024, 768
H, HD = 16, 48
SCALE = HD ** -0.5
NQB = N // 128      # 8 query blocks of 128
NKC = N // 128      # 8 key chunks of 128
NCC = C // 128      # 6 channel chunks
NPAIR = H // 2      # 8 head pairs
NTAU = 16           # q8-subblocks per 128-query block
bf16 = ml_dtypes.bfloat16


def _host_consts(w_qkv, b_qkv, w_l, b_l, w_w, b_w, w_proj, b_proj):
    """Build all weight/constant arrays in the exact SBUF layouts the kernel
    DMAs them into. All padded layouts use head-pair tiles with heads at
    partition offsets 0 and 64 (48 rows each)."""
    del b_l  # softmax shift invariant
    f32 = np.float32
    # padded-column qkv weight: wqkvT[c, sec*1024 + h*64 + d] = w_qkv[sec*768 + h*48 + d, c]
    wqkvT = np.zeros((C, 3 * 1024), f32)
    for sec in range(3):
        for h in range(H):
            cols = w_qkv[sec * C + h * HD:sec * C + (h + 1) * HD, :]   # [48, 768]
            wqkvT[:, sec * 1024 + h * 64:sec * 1024 + h * 64 + HD] = cols.T
    wqkvT = wqkvT.astype(bf16)

    # padded per-head-pair biases [128, 8] for q (scaled), k, v
    def pad_bias(vec, scale=1.0):
        out = np.zeros((128, NPAIR), f32)
        for t in range(NPAIR):
            for par in range(2):
                h = 2 * t + par
                out[64 * par:64 * par + HD, t] = vec[h * HD:(h + 1) * HD] * scale
        return out

    bq = pad_bias(b_qkv[0:C], SCALE)
    bk = pad_bias(b_qkv[C:2 * C])
    bv = pad_bias(b_qkv[2 * C:3 * C])

    # block-diagonal mix matrices [128,128]: BD[q8*16+h, q8*16+g] = w[g, h]
    def blockdiag(w):
        out = np.zeros((128, 128), f32)
        for q8 in range(8):
            out[q8 * 16:(q8 + 1) * 16, q8 * 16:(q8 + 1) * 16] = w.T
        return out

    bdl = blockdiag(w_l).astype(bf16)
    bdw = blockdiag(w_w)                                        # f32

    # b_w broadcast for the OAT correction: bwpad[p, t] = b_w[head]
    bwpad = np.zeros((128, NPAIR), f32)
    for t in range(NPAIR):
        for par in range(2):
            bwpad[64 * par:64 * par + HD, t] = b_w[2 * t + par]

    # w_proj^T with v-padded rows: [1024, 768], row h*64+d = w_proj[:, h*48+d]
    wprojT = np.zeros((1024, C), f32)
    for h in range(H):
        wprojT[h * 64:h * 64 + HD, :] = w_proj[:, h * HD:(h + 1) * HD].T
    wprojT = wprojT.astype(bf16)

    bproj = np.ascontiguousarray(b_proj.reshape(NCC, 128).T).astype(f32)  # [128, 6]
    ident = np.eye(128, dtype=bf16)

    return {
        "wqkvT": wqkvT, "bq": bq, "bk": bk, "bv": bv,
        "bdl": bdl, "bdw": bdw, "bwpad": bwpad,
        "wprojT": wprojT, "bproj": bproj, "ident": ident,
    }


@functools.lru_cache(maxsize=1)
def _build():
    PHASES = int(os.environ.get("KBUILD_PHASES", "3"))
    P2SUB = int(os.environ.get("KBUILD_P2SUB", "3"))
    import concourse.bass as bass
    import concourse.mybir as mybir
    import concourse.tile as tile
    from concourse import bacc
    from contextlib import ExitStack

    dt = mybir.dt
    AF = mybir.ActivationFunctionType

    nc = bacc.Bacc(trn_type="TRN2")

    x_ext = nc.declare_dram_parameter("x", [N, C], dt.float32, isOutput=False)
    wqkvT_ext = nc.declare_dram_parameter("wqkvT", [C, 3 * 1024], dt.bfloat16, isOutput=False)
    bq_ext = nc.declare_dram_parameter("bq", [128, NPAIR], dt.float32, isOutput=False)
    bk_ext = nc.declare_dram_parameter("bk", [128, NPAIR], dt.float32, isOutput=False)
    bv_ext = nc.declare_dram_parameter("bv", [128, NPAIR], dt.float32, isOutput=False)
    bdl_ext = nc.declare_dram_parameter("bdl", [128, 128], dt.bfloat16, isOutput=False)
    bdw_ext = nc.declare_dram_parameter("bdw", [128, 128], dt.float32, isOutput=False)
    bwpad_ext = nc.declare_dram_parameter("bwpad", [128, NPAIR], dt.float32, isOutput=False)
    wprojT_ext = nc.declare_dram_parameter("wprojT", [1024, C], dt.bfloat16, isOutput=False)
    bproj_ext = nc.declare_dram_parameter("bproj", [128, NCC], dt.float32, isOutput=False)
    ident_ext = nc.declare_dram_parameter("ident", [128, 128], dt.bfloat16, isOutput=False)
    outT_ext = nc.declare_dram_parameter("outT", [C, N], dt.float32, isOutput=True)

    with tile.TileContext(nc) as tc, ExitStack() as top:
        const = top.enter_context(tc.tile_pool(name="const", bufs=1))

        # ---- persistent constants ----
        ident = const.tile([128, 128], dt.bfloat16, tag="ident")
        nc.sync.dma_start(out=ident, in_=ident_ext[:])
        bdl = const.tile([128, 128], dt.bfloat16, tag="bdl")
        nc.sync.dma_start(out=bdl, in_=bdl_ext[:])
        bdw = const.tile([128, 128], dt.float32, tag="bdw")
        nc.sync.dma_start(out=bdw, in_=bdw_ext[:])
        bq = const.tile([128, NPAIR], dt.float32, tag="bq")
        nc.sync.dma_start(out=bq, in_=bq_ext[:])
        bk = const.tile([128, NPAIR], dt.float32, tag="bk")
        nc.sync.dma_start(out=bk, in_=bk_ext[:])
        bv = const.tile([128, NPAIR], dt.float32, tag="bv")
        nc.sync.dma_start(out=bv, in_=bv_ext[:])
        bwpad = const.tile([128, NPAIR], dt.float32, tag="bwpad")
        nc.sync.dma_start(out=bwpad, in_=bwpad_ext[:])
        bproj = const.tile([128, NCC], dt.float32, tag="bproj")
        nc.sync.dma_start(out=bproj, in_=bproj_ext[:])
        wproj_sb = []
        for kk in range(8):
            t = const.tile([128, C], dt.bfloat16, tag=f"wproj{kk}")
            nc.sync.dma_start(out=t, in_=wprojT_ext[kk * 128:(kk + 1) * 128, :])
            wproj_sb.append(t)

        # persistent activations
        qT = [const.tile([48, N], dt.bfloat16, tag=f"qT{t}", name=f"qT{t}") for t in range(H)]
        kT = [const.tile([48, N], dt.bfloat16, tag=f"kT{t}", name=f"kT{t}") for t in range(H)]
        vtok = [const.tile([128, H * 64], dt.bfloat16, tag=f"vtok{kc}", name=f"vtok{kc}") for kc in range(NKC)]
        oat = [const.tile([128, N], dt.bfloat16, tag=f"oat{t}", name=f"oat{t}") for t in range(NPAIR)]
        corr = const.tile([128, NPAIR], dt.float32, tag="corr")
        for t in range(NPAIR):
            nc.gpsimd.memset(oat[t][32:64, :], 0.0)
            nc.gpsimd.memset(oat[t][96:128, :], 0.0)

        # ================= PHASE 1: qkv projections =================
        with ExitStack() as ph1:
            p1 = ph1.enter_context(tc.tile_pool(name="ph1", bufs=3))
            p1w = ph1.enter_context(tc.tile_pool(name="ph1w", bufs=1))
            ps_qkv = ph1.enter_context(tc.tile_pool(name="ps_qkv", bufs=4, space="PSUM"))
            ps_tr1 = ph1.enter_context(tc.tile_pool(name="ps_tr1", bufs=4, space="PSUM"))

            wqkv_sb = []
            for cc in range(NCC):
                t = p1w.tile([128, 3 * 1024], dt.bfloat16, tag=f"wqkv{cc}")
                nc.sync.dma_start(out=t, in_=wqkvT_ext[cc * 128:(cc + 1) * 128, :])
                wqkv_sb.append(t)
            xT = [p1w.tile([128, N], dt.bfloat16, tag=f"xT{cc}", name=f"xT{cc}") for cc in range(NCC)]
            vT = [p1w.tile([128, N], dt.bfloat16, tag=f"vT{t}", name=f"vT{t}") for t in range(NPAIR)]
            for t in range(NPAIR):
                nc.gpsimd.memset(vT[t][32:64, :], 0.0)
                nc.gpsimd.memset(vT[t][96:128, :], 0.0)

            # x -> xT (transpose to channel-major), bf16
            for qb in range(NQB):
                xs = p1.tile([128, C], dt.float32, tag="xs")
                nc.sync.dma_start(out=xs, in_=x_ext[qb * 128:(qb + 1) * 128, :])
                xb = p1.tile([128, C], dt.bfloat16, tag="xb")
                nc.scalar.copy(xb, xs)
                for cc in range(NCC):
                    tp = ps_tr1.tile([128, 128], dt.bfloat16, tag="tr")
                    nc.tensor.transpose(tp, xb[:, cc * 128:(cc + 1) * 128], ident)
                    eng = nc.scalar if (cc % 2) else nc.vector
                    if eng is nc.scalar:
                        nc.scalar.copy(xT[cc][:, qb * 128:(qb + 1) * 128], tp)
                    else:
                        nc.vector.tensor_copy(xT[cc][:, qb * 128:(qb + 1) * 128], tp)

            # q, k, v projections in M=96 head-pair groups
            def qkv_group(col0, dst_of_par, bias, bcol, scale):
                for nn in range(2):
                    ps = ps_qkv.tile([128, 512], dt.float32, tag="qkv")
                    for cc in range(NCC):
                        nc.tensor.matmul(
                            ps,
                            lhsT=wqkv_sb[cc][:, col0:col0 + 128],
                            rhs=xT[cc][:, nn * 512:(nn + 1) * 512],
                            start=(cc == 0), stop=(cc == NCC - 1),
                        )
                    for par in range(2):
                        nc.scalar.activation(
                            dst_of_par(par)[:, nn * 512:(nn + 1) * 512],
                            ps[64 * par:64 * par + HD, :],
                            AF.Identity,
                            bias=bias[64 * par:64 * par + HD, bcol:bcol + 1],
                            scale=scale,
                        )

            for g in range(NPAIR):
                qkv_group(g * 128, lambda par, g=g: qT[2 * g + par][0:HD, :], bq, g, float(SCALE))
                qkv_group(1024 + g * 128, lambda par, g=g: kT[2 * g + par][0:HD, :], bk, g, 1.0)
                qkv_group(2048 + g * 128, lambda par, g=g: vT[g][64 * par:64 * par + HD, :], bv, g, 1.0)

            # vT -> v token-major (padded cols h*64+d), and colsum correction
            vsum = p1.tile([128, NPAIR], dt.float32, tag="vsum")
            for t in range(NPAIR):
                nc.vector.reduce_sum(vsum[:, t:t + 1], vT[t][:, :], axis=mybir.AxisListType.X)
                for kc in range(NKC):
                    tp = ps_tr1.tile([128, 128], dt.bfloat16, tag="tr")
                    nc.tensor.transpose(tp, vT[t][:, kc * 128:(kc + 1) * 128], ident)
                    if t % 2:
                        nc.scalar.copy(vtok[kc][:, t * 128:(t + 1) * 128], tp)
                    else:
                        nc.vector.tensor_copy(vtok[kc][:, t * 128:(t + 1) * 128], tp)
            nc.vector.tensor_mul(corr, vsum, bwpad)

        # ================= PHASE 2: attention =================
        if PHASES == 1:
            for cg in range(NCC):
                nc.gpsimd.dma_start(out=outT_ext[cg * 128:(cg + 1) * 128, :], in_=vtok[cg][:, 0:1024])
            nc.compile_tc = None
        ph2 = top.enter_context(ExitStack())
        if PHASES >= 2:
            _run_phase2 = True
        else:
            _run_phase2 = False
        st_pool = ph2.enter_context(tc.tile_pool(name="st", bufs=8))
        pt_pool = ph2.enter_context(tc.tile_pool(name="pt", bufs=8))
        sm_pool = ph2.enter_context(tc.tile_pool(name="sm", bufs=2))
        tiny_pool = ph2.enter_context(tc.tile_pool(name="tiny", bufs=4))
        ps_big = ph2.enter_context(tc.tile_pool(name="ps_big", bufs=2, space="PSUM"))
        ps_med = ph2.enter_context(tc.tile_pool(name="ps_med", bufs=2, space="PSUM"))
        ps_sm = ph2.enter_context(tc.tile_pool(name="ps_sm", bufs=2, space="PSUM"))

        NQB_RUN = int(os.environ.get("KBUILD_NQB", str(NQB)))
        for Q in range(NQB_RUN if _run_phase2 else 0):
            # ---- scores (transposed): ST_kc[k, h*128+q] ----
            st = [st_pool.tile([128, H * 128], dt.bfloat16, tag="st", name=f"stQ{Q}_{i}") for i in range(NKC)]
            NKC_RUN = int(os.environ.get("KBUILD_NKC", str(NKC)))
            for kc in range(NKC_RUN if P2SUB >= 1 else 0):
                for half in range(2):
                    ps = ps_big.tile([128, 1024], dt.float32, tag="big")
                    for hh in range(8):
                        h = half * 8 + hh
                        pair, par = h // 2, h % 2
                        nc.tensor.matmul(
                            ps[:, hh * 128:(hh + 1) * 128],
                            lhsT=kT[h][0:HD, kc * 128:(kc + 1) * 128],
                            rhs=qT[h][0:HD, Q * 128:(Q + 1) * 128],
                            start=True, stop=True,
                        )
                    base = st[kc]
                    if os.environ.get("KBUILD_NOSTRIDE"):
                        dst = base[:, half * 1024:(half + 1) * 1024]
                    else:
                        dst = bass.AP(
                            tensor=base.tensor,
                            offset=base.offset + half * 8,
                            ap=[base.ap[0], [1, 8], [16, 128]],
                        )
                    if half:
                        nc.scalar.copy(dst, ps)
                    else:
                        nc.vector.tensor_copy(dst, ps)

            # ---- per-tau: gather-transpose, mix1, softmax, fused mix2+transpose ----
            pt = [pt_pool.tile([128, NTAU * 128], dt.bfloat16, tag="pt", name=f"ptQ{Q}_{i}") for i in range(NKC)]
            for tau in range(NTAU if P2SUB >= 2 else 0):
                shat = sm_pool.tile([128, 1024], dt.bfloat16, tag="shat")
                for grp in range(2):
                    ps = ps_med.tile([128, 512], dt.bfloat16, tag="med")
                    for j in range(4):
                        kc = grp * 4 + j
                        nc.tensor.transpose(
                            ps[:, j * 128:(j + 1) * 128],
                            st[kc][:, tau * 128:(tau + 1) * 128], ident)
                    dst = shat[:, grp * 512:(grp + 1) * 512]
                    if grp:
                        nc.scalar.copy(dst, ps)
                    else:
                        nc.vector.tensor_copy(dst, ps)

                tmix = ps_big.tile([128, 1024], dt.float32, tag="big")
                for nn in range(2):
                    nc.tensor.matmul(
                        tmix[:, nn * 512:(nn + 1) * 512],
                        lhsT=bdl, rhs=shat[:, nn * 512:(nn + 1) * 512],
                        start=True, stop=True,
                    )
                att = sm_pool.tile([128, 1024], dt.bfloat16, tag="att")
                ssum = tiny_pool.tile([128, 1], dt.float32, tag="ssum")
                nc.scalar.activation(att, tmix, AF.Exp, accum_out=ssum)
                recip = tiny_pool.tile([128, 1], dt.float32, tag="recip")
                nc.vector.reciprocal(recip, ssum)
                bdws = tiny_pool.tile([128, 128], dt.bfloat16, tag="bdws")
                nc.vector.tensor_scalar_mul(bdws, bdw, recip)

                # fused mix2 + transpose: P^T chunk = att_chunk.T @ bdws
                for kc in range(NKC):
                    pps = ps_sm.tile([128, 128], dt.float32, tag="sm")
                    nc.tensor.matmul(
                        pps, lhsT=att[:, kc * 128:(kc + 1) * 128], rhs=bdws,
                        start=True, stop=True,
                    )
                    base = pt[kc]
                    dst = bass.AP(
                        tensor=base.tensor,
                        offset=base.offset + tau * 8,
                        ap=[base.ap[0], [1, 8], [128, 16]],
                    )
                    if kc % 2:
                        nc.scalar.copy(dst, pps)
                    else:
                        nc.vector.tensor_copy(dst, pps)

            # ---- attn @ V, transpose out to d-major OAT ----
            for h in range(H if P2SUB >= 3 else 0):
                pair, par = h // 2, h % 2
                ops = ps_sm.tile([128, 128], dt.float32, tag="sm")
                for kc in range(NKC):
                    nc.tensor.matmul(
                        ops[:, 0:HD],
                        lhsT=pt[kc][:, h * 128:(h + 1) * 128],
                        rhs=vtok[kc][:, h * 64:h * 64 + HD],
                        start=(kc == 0), stop=(kc == NKC - 1),
                    )
                oasb = sm_pool.tile([128, HD], dt.bfloat16, tag="oasb")
                nc.scalar.copy(oasb, ops[:, 0:HD])
                otp = ps_med.tile([128, 128], dt.bfloat16, tag="med")
                nc.tensor.transpose(otp[0:HD, :], oasb, ident)
                nc.scalar.activation(
                    oat[pair][64 * par:64 * par + HD, Q * 128:(Q + 1) * 128],
                    otp[0:HD, :],
                    AF.Identity,
                    bias=corr[64 * par:64 * par + HD, pair:pair + 1],
                )

        # ================= PHASE 3: output projection (transposed) =================
        if PHASES == 2:
            for cg in range(NCC):
                nc.gpsimd.dma_start(out=outT_ext[cg * 128:(cg + 1) * 128, :], in_=oat[cg])
        out_pool = top.enter_context(tc.tile_pool(name="outp", bufs=2))
        for cg in range(NCC if PHASES >= 3 else 0):
            ps = ps_big.tile([128, 1024], dt.float32, tag="big")
            for nn in range(2):
                for kk in range(8):
                    nc.tensor.matmul(
                        ps[:, nn * 512:(nn + 1) * 512],
                        lhsT=wproj_sb[kk][:, cg * 128:(cg + 1) * 128],
                        rhs=oat[kk][:, nn * 512:(nn + 1) * 512],
                        start=(kk == 0), stop=(kk == 7),
                    )
            fout = out_pool.tile([128, N], dt.float32, tag="fout")
            nc.scalar.activation(fout, ps, AF.Identity, bias=bproj[:, cg:cg + 1])
            nc.sync.dma_start(out=outT_ext[cg * 128:(cg + 1) * 128, :], in_=fout)

    nc.compile()
    return nc


LAST_RESULTS = None


def kernel(x, w_qkv, b_qkv, w_l, b_l, w_w, b_w, w_proj, b_proj):
    global LAST_RESULTS
    from concourse.bass_utils import run_bass_kernel_spmd

    x = np.asarray(x, np.float32)
    consts = _host_consts(
        np.asarray(w_qkv, np.float32), np.asarray(b_qkv, np.float32),
        np.asarray(w_l, np.float32), np.asarray(b_l, np.float32),
        np.asarray(w_w, np.float32), np.asarray(b_w, np.float32),
        np.asarray(w_proj, np.float32), np.asarray(b_proj, np.float32),
    )
    nc = _build()
    in_maps = [dict(consts, x=np.ascontiguousarray(x[i])) for i in range(B)]
    res = run_bass_kernel_spmd(nc, in_maps, core_ids=list(range(B)))
    LAST_RESULTS = res
    out = np.stack([np.ascontiguousarray(res.results[i]["outT"].T) for i in range(B)])
    return out.astype(np.float32)
